# revision 40
# baseline (speedup 1.0000x reference)
"""AFMoE attention layer on 8 NeuronCores (Trainium2, Bass/Tile).

Sharding: core c = (batch b = c//4) x (kv-head group g = c%4).
Each core computes its batch's q-heads 4g..4g+3 + kv head g end-to-end and a
partial output y_c = O_gated @ Wo[:, 512g:512(g+1)].T; the host sums the 4
group partials per batch (row-parallel Wo reduction done on host).

v2: bf16 operands on the PE (same 1 cycle/row as f32r, half the DMA/SBUF),
gate projection fused into the main projection loop (no second read of x),
software-pipelined attention (scores of task i+1 issued before PV of task i
so the PE never waits on the scalar-engine exp), sliding-window masking via
gpsimd affine_select on the exp'd scores (no mask matmuls, no mask DMA),
softmax denominator broadcast via gpsimd partition_broadcast, Wo matmuls
interleaved into the attention pipeline as PE filler, output DMA'd straight
from PSUM.
"""
import os

import ml_dtypes
import numpy as np

import concourse.bass as bass
import concourse.mybir as mybir
import concourse.tile as tile
from concourse.bass_utils import run_bass_kernel_spmd
from concourse.masks import make_identity

F32 = mybir.dt.float32
F32R = mybir.dt.float32r
BF16 = mybir.dt.bfloat16
AF = mybir.ActivationFunctionType
ALU = mybir.AluOpType
AX = mybir.AxisListType

B, S, H = 2, 2048, 2048
NH, NKV, D = 16, 4, 128
GROUPS = NH // NKV          # q heads per kv head = 4
QH = GROUPS                 # per-core q heads
DQ = QH * D                 # 512
EPS = 1e-5
NT = S // 128               # 16 s-tiles
NP = NT // 2                # 8 s-pairs (256 q rows each)
HC = H // 128               # 16 h-chunks
LAM = float(D) ** -0.5
NW = DQ + 2 * D             # 768
G = 2                       # k-blocks per PSUM score group (1 bank)

_nsplit = [0]


def _split_excess_waits(nc, limit=1):
    """This walrus build accepts only one semaphore wait per instruction
    (fp32/fp32r matmuls included). Move excess waits onto preceding
    same-engine NoOps; engine program order keeps this correct."""
    import bass_rust
    for blk in nc.m.functions[0].blocks:
        lst = blk.instructions
        idx = 0
        while idx < len(lst):
            inst = lst[idx]
            si = inst.sync_info
            if (si is None or len(si.on_wait) <= limit
                    or type(inst).__name__ == "InstCollectiveCompute"
                    or inst.engine == mybir.EngineType.Unassigned):
                idx += 1
                continue
            waits = list(si.on_wait)
            kept, excess = waits[-limit:], waits[:-limit]
            new_insts = []
            for w in excess:
                _nsplit[0] += 1
                nop = mybir.InstNoOp(name=f"WS-{_nsplit[0]}", ins=[], outs=[])
                nop.engine = inst.engine
                nop.sync_info = bass_rust.SyncInfo(on_wait=[w], on_update=[])
                new_insts.append(nop)
            inst.sync_info = bass_rust.SyncInfo(on_wait=kept,
                                                on_update=list(si.on_update))
            lst[idx:idx] = new_insts
            idx += len(new_insts) + 1


def _mask_plan(mask2d):
    """Classify the additive mask in [256(q) x 128(k)] slabs.

    Returns rows: rows[pair] = list of (kj, cuts) over a contiguous kj
    range, where cuts is a tuple of ('ge'|'le', base) affine-select specs
    in the transposed [k(part) x q(free)] layout: keep iff
    f - p + base {>=,<=} 0 for every cut.
    """
    rows = []
    fidx = np.arange(256)[None, :]
    pidx = np.arange(128)[:, None]
    diff = fidx - pidx                       # [128, 256]
    for p in range(NP):
        qsl = slice(p * 256, (p + 1) * 256)
        live = [kj for kj in range(NT)
                if (mask2d[qsl, kj * 128:(kj + 1) * 128] == 0.0).any()]
        if not live:
            raise ValueError("fully-masked query row block unsupported")
        lo, hi = min(live), max(live)
        row = []
        for kj in range(lo, hi + 1):
            blk = mask2d[qsl, kj * 128:(kj + 1) * 128]    # [256 q, 128 k]
            allowed = (blk == 0.0).T                      # [128 k, 256 q]
            if allowed.all():
                row.append((kj, ()))
                continue
            # allowed must be a function of (f - p) forming an interval
            dvals = np.unique(diff)
            adiff = {}
            ok = True
            for d in dvals:
                sel = allowed[diff == d]
                if sel.all():
                    adiff[d] = True
                elif not sel.any():
                    adiff[d] = False
                else:
                    ok = False
                    break
            live_d = sorted(d for d, a in adiff.items() if a)
            if not ok or not live_d:
                raise ValueError(f"non-affine mask block pair={p} kj={kj}")
            dlo, dhi = live_d[0], live_d[-1]
            if live_d != list(range(dlo, dhi + 1)):
                raise ValueError(f"non-interval mask block pair={p} kj={kj}")
            cuts = []
            if dlo > int(dvals.min()):
                cuts.append(('ge', -int(dlo)))
            if dhi < int(dvals.max()):
                cuts.append(('le', -int(dhi)))
            # verify
            keep = np.ones_like(allowed)
            for op, base in cuts:
                keep &= (diff + base >= 0) if op == 'ge' else (diff + base <= 0)
            if not (keep == allowed).all():
                raise ValueError(f"mask verify failed pair={p} kj={kj}")
            row.append((kj, tuple(cuts)))
        rows.append(row)
    return rows


def _build(rows):
    nc = bass.Bass()
    xt = nc.declare_dram_parameter("xt", [H, S], BF16, isOutput=False)
    wqkv = nc.declare_dram_parameter("wqkv", [H, NW], BF16, isOutput=False)
    wg = nc.declare_dram_parameter("wg", [H, DQ], BF16, isOutput=False)
    wo = nc.declare_dram_parameter("wo", [DQ, H], BF16, isOutput=False)
    cwq = nc.declare_dram_parameter("cwq", [S, D], F32, isOutput=False)
    swq = nc.declare_dram_parameter("swq", [S, D], F32, isOutput=False)
    cwk = nc.declare_dram_parameter("cwk", [S, D], F32, isOutput=False)
    swk = nc.declare_dram_parameter("swk", [S, D], F32, isOutput=False)
    y = nc.declare_dram_parameter("y", [S, H], F32, isOutput=True)
    dbg = bool(os.environ.get("KDBG"))
    if dbg:
        dbg_qT = nc.declare_dram_parameter("dbg_qT", [128, QH * S], BF16,
                                           isOutput=True)
        dbg_kT = nc.declare_dram_parameter("dbg_kT", [128, S], BF16,
                                           isOutput=True)
        dbg_v = nc.declare_dram_parameter("dbg_v", [128, NT * D], BF16,
                                          isOutput=True)
        dbg_sig = nc.declare_dram_parameter("dbg_sig", [128, QH * S], F32,
                                            isOutput=True)
        dbg_otg = nc.declare_dram_parameter("dbg_otg", [128, QH * S], BF16,
                                            isOutput=True)

    with tile.TileContext(nc) as tc, \
            nc.allow_low_precision(reason="bf16/fp32r matmul operands"), \
            tc.tile_pool(name="const", bufs=1) as const, \
            tc.tile_pool(name="persist", bufs=1) as pp, \
            tc.tile_pool(name="atw", bufs=1) as atw:
        identity_f = const.tile([128, 128], F32)
        make_identity(nc, identity_f)
        identity_r = const.tile([128, 128], F32R)
        nc.vector.tensor_copy(identity_r, identity_f)
        ones_mat_f = const.tile([128, 128], F32)
        nc.vector.memset(ones_mat_f, 1.0)
        ones_mat = const.tile([128, 128], BF16)
        nc.vector.tensor_copy(ones_mat, ones_mat_f)
        eps_t = const.tile([128, 1], F32)
        nc.vector.memset(eps_t, EPS)
        eps_dummy = const.tile([128, 1], F32)

        qT_all = pp.tile([128, QH, S], BF16)     # [d, h, s]
        kT_all = pp.tile([128, S], BF16)         # [d, s]
        v_all = pp.tile([128, NT, D], BF16)      # [s-part, s-tile, d]
        sigT_all = pp.tile([128, QH, S], F32)    # [d, m, s]
        OTg_all = pp.tile([128, QH, S], BF16)    # [d, h, s]
        wo_sb = atw.tile([128, QH, H], BF16)

        # ---------------- phase P-a: qkv + gate projections ---------------
        with tc.tile_pool(name="pw", bufs=1) as pw, \
                tc.tile_pool(name="pa", bufs=2) as pa, \
                tc.tile_pool(name="psa", bufs=1, space="PSUM") as psa:
            wqkv_sb = pw.tile([128, HC, NW], BF16)
            wg_sb = pw.tile([128, HC, DQ], BF16)
            xt4 = xt.rearrange("(c p) (t q) -> p c t q", p=128, q=256)

            def load_xt(pr, nsplit=4, eng=None):
                eng = eng or nc.sync
                t = pa.tile([128, HC, 256], BF16, tag="xt", bufs=2,
                            name=f"xt_p{pr}")
                for cq in range(HC // nsplit):
                    c0 = nsplit * cq
                    eng.dma_start(out=t[:, c0:c0 + nsplit, :],
                                  in_=xt4[:, c0:c0 + nsplit, pr, :])
                return t

            # Startup is SP-sequencer issue-rate bound (~565ns per
            # dma_start), so spread the first-wave issues across the idle
            # vector/scalar DGE queues: wqkv on sync, pair-0 x on vector,
            # wg on scalar.  wo is trickled in later, mid-loop.
            for h in range(HC):
                nc.sync.dma_start(out=wqkv_sb[:, h, :],
                                  in_=wqkv[h * 128:(h + 1) * 128, :])
            xt_next = load_xt(0, eng=nc.scalar)
            for h in range(HC):
                nc.scalar.dma_start(out=wg_sb[:, h, :],
                                    in_=wg[h * 128:(h + 1) * 128, :])

            for pr in range(NP):
                xt_t = xt_next
                ropes = {}
                for st in (2 * pr, 2 * pr + 1):
                    half = slice((st % 2) * 128, (st % 2) * 128 + 128)
                    sl = slice(st * 128, (st + 1) * 128)
                    cwq_t = pa.tile([128, D], F32, tag="cwq", bufs=4)
                    swq_t = pa.tile([128, D], F32, tag="swq", bufs=4)
                    cwk_t = pa.tile([128, D], F32, tag="cwk", bufs=4)
                    swk_t = pa.tile([128, D], F32, tag="swk", bufs=4)
                    nc.sync.dma_start(out=cwq_t, in_=cwq[sl, :])
                    nc.sync.dma_start(out=swq_t, in_=swq[sl, :])
                    nc.sync.dma_start(out=cwk_t, in_=cwk[sl, :])
                    nc.sync.dma_start(out=swk_t, in_=swk[sl, :])

                    pqkv = psa.tile([128, NW], F32, tag="pqkv", bufs=2)
                    for h in range(HC):
                        nc.tensor.matmul(pqkv[:, :DQ], xt_t[:, h, half],
                                         wqkv_sb[:, h, :DQ],
                                         start=(h == 0), stop=(h == HC - 1))
                        nc.tensor.matmul(pqkv[:, DQ:], xt_t[:, h, half],
                                         wqkv_sb[:, h, DQ:],
                                         start=(h == 0), stop=(h == HC - 1))
                    q_raw = pa.tile([128, DQ], F32, tag="qraw")
                    nc.scalar.copy(q_raw, pqkv[:, :DQ])
                    k_raw = pa.tile([128, D], F32, tag="kraw")
                    nc.scalar.copy(k_raw, pqkv[:, DQ:DQ + D])
                    nc.scalar.copy(v_all[:, st, :], pqkv[:, DQ + D:])

                    sq = pa.tile([128, DQ], F32, tag="sq")
                    nc.vector.tensor_mul(sq, q_raw, q_raw)
                    sqk = pa.tile([128, D], F32, tag="sqk")
                    nc.vector.tensor_mul(sqk, k_raw, k_raw)
                    ss = pa.tile([128, QH + 1], F32, tag="ss")
                    nc.vector.tensor_reduce(
                        ss[:, :QH], sq.rearrange("p (h d) -> p h d", d=D),
                        axis=AX.X, op=ALU.add)
                    nc.vector.tensor_reduce(ss[:, QH:], sqk,
                                            axis=AX.X, op=ALU.add)
                    rt = pa.tile([128, QH + 1], F32, tag="rt")
                    nc.scalar.activation(rt, ss, AF.Sqrt, bias=eps_t,
                                         scale=1.0 / D)
                    rq = pa.tile([128, QH + 1], F32, tag="rq")
                    nc.vector.reciprocal(rq, rt)
                    rk = rq[:, QH:QH + 1]

                    # rope swaps (half-rotations) of the raw values, on DVE
                    # (gpsimd CASTs are ~3x slower and add engine hops)
                    r_q = pa.tile([128, QH, D], F32R, tag="rqrot")
                    qv = q_raw.rearrange("p (h s d) -> p h s d", h=QH, s=2)
                    rv = r_q.rearrange("p h (s d) -> p h s d", s=2)
                    nc.vector.tensor_copy(rv[:, :, 0, :], qv[:, :, 1, :])
                    nc.vector.tensor_copy(rv[:, :, 1, :], qv[:, :, 0, :])
                    r_k = pa.tile([128, D], F32R, tag="rkrot")
                    nc.vector.tensor_copy(r_k[:, :64], k_raw[:, 64:])
                    nc.vector.tensor_copy(r_k[:, 64:], k_raw[:, :64])

                    qrope = pa.tile([128, QH, D], F32R, tag="qrope", bufs=3)
                    qh = q_raw.rearrange("p (h d) -> p h d", d=D)
                    for h in range(QH):
                        nc.vector.scalar_tensor_tensor(
                            qrope[:, h, :], qh[:, h, :], rq[:, h:h + 1], cwq_t,
                            op0=ALU.mult, op1=ALU.mult)
                        nc.vector.scalar_tensor_tensor(
                            r_q[:, h, :], r_q[:, h, :], rq[:, h:h + 1], swq_t,
                            op0=ALU.mult, op1=ALU.mult)
                    nc.gpsimd.tensor_tensor(qrope, qrope, r_q, op=ALU.add)

                    krope = pa.tile([128, D], F32R, tag="krope", bufs=3)
                    nc.vector.scalar_tensor_tensor(krope, k_raw, rk, cwk_t,
                                                   op0=ALU.mult, op1=ALU.mult)
                    nc.vector.scalar_tensor_tensor(r_k, r_k, rk, swk_t,
                                                   op0=ALU.mult, op1=ALU.mult)
                    nc.gpsimd.tensor_tensor(krope, krope, r_k, op=ALU.add)
                    ropes[st] = (qrope, krope)

                # prefetch next pair's x while this pair computes
                if pr + 1 < NP:
                    xt_next = load_xt(pr + 1)
                # trickle in wo during the middle of P-a
                if 2 <= pr <= 5:
                    dc = pr - 2
                    nc.sync.dma_start(out=wo_sb[:, dc, :H // 2],
                                      in_=wo[dc * 128:(dc + 1) * 128, :H // 2])
                    nc.sync.dma_start(out=wo_sb[:, dc, H // 2:],
                                      in_=wo[dc * 128:(dc + 1) * 128, H // 2:])

                def emit_transpose(st):
                    qrope, krope = ropes.pop(st)
                    sl = slice(st * 128, (st + 1) * 128)
                    ptqk = psa.tile([128, 5, 128], F32R, tag="pt", bufs=1)
                    for h in range(QH):
                        nc.tensor.transpose(ptqk[:, h, :], qrope[:, h, :],
                                            identity_r)
                    nc.tensor.transpose(ptqk[:, QH, :], krope, identity_r)
                    nc.scalar.copy(qT_all[:, :, sl], ptqk[:, :QH, :])
                    nc.scalar.copy(kT_all[:, sl], ptqk[:, QH, :])

                # PE order: qkv(st0) qkv(st1) gate transp(st0) transp(st1);
                # the ~7us rope chain (DVE/gpsimd) hides under qkv+gate.
                pg = psa.tile([128, QH, 256], F32, tag="pg", bufs=1)
                for m in range(QH):
                    for h in range(HC):
                        nc.tensor.matmul(
                            pg[:, m, :], wg_sb[:, h, m * 128:(m + 1) * 128],
                            xt_t[:, h, :],
                            start=(h == 0), stop=(h == HC - 1))
                qsl = slice(pr * 256, (pr + 1) * 256)
                nc.scalar.activation(sigT_all[:, :, qsl], pg, AF.Sigmoid)
                # dummy op to preload the sqrt act table off the rope
                # critical path (sqrt and sigmoid live in different tables)
                nc.scalar.activation(eps_dummy, eps_t, AF.Sqrt)

                emit_transpose(2 * pr)
                emit_transpose(2 * pr + 1)

        # ---------------- attention + gating + Wo -------------------------
        with tc.tile_pool(name="at", bufs=1) as at, \
                tc.tile_pool(name="ps_st", bufs=1, space="PSUM") as ps_st, \
                tc.tile_pool(name="ps_ot", bufs=1, space="PSUM") as ps_ot, \
                tc.tile_pool(name="ps_y", bufs=1, space="PSUM") as ps_y:
            # big pairs first to saturate the pipeline during warmup, but
            # end on big pairs too so the wo backlog drains before the tail
            pair_order = [NP - 1, NP - 2] + list(range(NP - 2))
            tasks = []
            for pr in pair_order:
                row = rows[pr]
                groups = [row[i:i + G] for i in range(0, len(row), G)]
                for h in range(QH):
                    for gi, grp in enumerate(groups):
                        tasks.append((pr, h, grp, gi == 0,
                                      gi == len(groups) - 1))

            ot_tiles = {}
            rs_tiles = {}
            est_tiles = {}
            wo_queue = []          # (ready_idx, st, n4)

            def emit_scores(idx, t):
                pr, h, grp, first, last = t
                qsl = slice(pr * 256, (pr + 1) * 256)
                ng = len(grp)
                st_ps = ps_st.tile([128, G, 256], F32, tag="st", bufs=3)
                for j, (kj, cuts) in enumerate(grp):
                    nc.tensor.matmul(
                        st_ps[:, j, :],
                        kT_all[:, kj * 128:(kj + 1) * 128],
                        qT_all[:, h, qsl],
                        start=True, stop=True)
                est = at.tile([128, G, 256], BF16, tag="est", bufs=4)
                nc.scalar.activation(
                    est[:, :ng, :].rearrange("p g q -> p (g q)"),
                    st_ps[:, :ng, :].rearrange("p g q -> p (g q)"),
                    AF.Exp)
                for j, (kj, cuts) in enumerate(grp):
                    for op, base in cuts:
                        # codegen only implements is_ge; express 'le' as the
                        # negated iota: f-p+base<=0  <=>  -f+p-base>=0
                        if op == 'ge':
                            pat, cm, b = [[1, 256]], -1, base
                        else:
                            pat, cm, b = [[-1, 256]], 1, -base
                        nc.gpsimd.affine_select(
                            est[:, j, :], est[:, j, :],
                            pattern=pat, compare_op=ALU.is_ge,
                            fill=0.0, base=b, channel_multiplier=cm)
                est_tiles[idx] = est

            def emit_pv(idx, t):
                pr, h, grp, first, last = t
                est = est_tiles.pop(idx)
                if first:
                    # ot and rowsum share one bank as ONE accumulation
                    # group: the first start=True zeroes the whole 2KB
                    # zero region (both halves), everything else joins
                    # with start=False
                    ot_tiles[(pr, h)] = ps_ot.tile([128, 512], F32,
                                                   tag="otrs", bufs=2,
                                                   name=f"otrs_{pr}_{h}")
                otrs = ot_tiles[(pr, h)]
                ot_t = otrs[:, 0:256]
                rs_t = otrs[:, 256:512]
                ng = len(grp)
                for j, (kj, cuts) in enumerate(grp):
                    stop = last and (j == ng - 1)
                    nc.tensor.matmul(ot_t, v_all[:, kj, :], est[:, j, :],
                                     start=(first and j == 0), stop=False)
                    # ones matrix: rowsum lands replicated on all partitions
                    nc.tensor.matmul(rs_t, ones_mat, est[:, j, :],
                                     start=False, stop=stop)

            def emit_norm(pr, h):
                qsl = slice(pr * 256, (pr + 1) * 256)
                otrs = ot_tiles.pop((pr, h))
                ot_t = otrs[:, 0:256]
                bcast = at.tile([128, 256], F32, tag="bcast", bufs=2)
                nc.vector.reciprocal(bcast, otrs[:, 256:512])
                sgr = at.tile([128, 256], F32, tag="sgr", bufs=2)
                nc.vector.tensor_mul(sgr, bcast, sigT_all[:, h, qsl])
                nc.vector.tensor_mul(OTg_all[:, h, qsl], ot_t, sgr)

            def emit_wo_chunk(st, n4):
                sl = slice(st * 128, (st + 1) * 128)
                py = ps_y.tile([128, 512], F32, tag="py", bufs=2)
                for dc in range(QH):
                    nc.tensor.matmul(
                        py, OTg_all[:, dc, sl],
                        wo_sb[:, dc, n4 * 512:(n4 + 1) * 512],
                        start=(dc == 0), stop=(dc == QH - 1))
                y_sb = at.tile([128, 512], F32, tag="ysb", bufs=2)
                nc.vector.tensor_copy(y_sb, py)
                nc.sync.dma_start(out=y[sl, n4 * 512:(n4 + 1) * 512], in_=y_sb)

            LOOKAHEAD = 3

            def finish(pidx, pt, idx):
                emit_pv(pidx, pt)
                pr, h, grp, first, last = pt
                if last:
                    emit_norm(pr, h)
                    if h == QH - 1:
                        for st in (2 * pr, 2 * pr + 1):
                            for n4 in range(4):
                                wo_queue.append((idx, st, n4))
                for _ in range(2):
                    if wo_queue and idx - wo_queue[0][0] >= 3:
                        _, st, n4 = wo_queue.pop(0)
                        emit_wo_chunk(st, n4)

            pend = []
            for idx, t in enumerate(tasks):
                emit_scores(idx, t)
                pend.append((idx, t))
                if len(pend) > LOOKAHEAD:
                    pidx, pt = pend.pop(0)
                    finish(pidx, pt, idx)
            for pidx, pt in pend:
                finish(pidx, pt, len(tasks))
            for _, st, n4 in wo_queue:
                emit_wo_chunk(st, n4)

            if dbg:
                for t, dst in [(qT_all, dbg_qT), (kT_all, dbg_kT),
                               (v_all, dbg_v), (sigT_all, dbg_sig),
                               (OTg_all, dbg_otg)]:
                    nc.sync.dma_start(out=dst[:, :], in_=t)

    _split_excess_waits(nc)
    return nc


_CACHE = {}
LAST_EXEC_TIME_NS = None
LAST_RESULTS = None


def _maybe_install_profile_hook():
    if not os.environ.get("BASS_TRACE"):
        return
    try:
        import sys
        import types
        import antenv
        if "antenv.axon_hooks" in sys.modules:
            return
        mod = types.ModuleType("antenv.axon_hooks")
        mod._hook = None
        mod.set_axon_ntff_profile_hook = lambda h: setattr(mod, "_hook", h)
        mod.get_axon_ntff_profile_hook = lambda: mod._hook
        sys.modules["antenv.axon_hooks"] = mod
        antenv.axon_hooks = mod
        from trn_agent_boot.trn_boot import _ntff_profile_via_ctypes
        mod.set_axon_ntff_profile_hook(
            _ntff_profile_via_ctypes("/opt/axon/libaxon_pjrt.so"))
    except Exception:
        pass


def kernel(hidden_states, cos, sin, attention_mask, Wq, Wk, Wv, Wo, Wg,
           q_norm_w, k_norm_w):
    global LAST_EXEC_TIME_NS, LAST_RESULTS
    _maybe_install_profile_hook()

    BF = ml_dtypes.bfloat16
    hidden_states = np.asarray(hidden_states, dtype=np.float32)
    cos = np.asarray(cos, dtype=np.float32)
    sin = np.asarray(sin, dtype=np.float32)
    mask2d = np.asarray(attention_mask, dtype=np.float32).reshape(S, S)
    Wq = np.asarray(Wq, dtype=np.float32)
    Wk = np.asarray(Wk, dtype=np.float32)
    Wv = np.asarray(Wv, dtype=np.float32)
    Wo = np.asarray(Wo, dtype=np.float32)
    Wg = np.asarray(Wg, dtype=np.float32)
    qw = np.asarray(q_norm_w, dtype=np.float32)
    kw = np.asarray(k_norm_w, dtype=np.float32)

    rows = _mask_plan(mask2d)
    plan_key = tuple(tuple(r) for r in rows)
    if plan_key not in _CACHE:
        _CACHE[plan_key] = _build(rows)
    nc = _CACHE[plan_key]

    sign = np.concatenate([-np.ones(D // 2), np.ones(D // 2)]).astype(np.float32)
    qw_swap = np.concatenate([qw[D // 2:], qw[:D // 2]])
    kw_swap = np.concatenate([kw[D // 2:], kw[:D // 2]])

    in_maps = []
    for c in range(8):
        b, g = divmod(c, 4)
        qs = slice(g * DQ, (g + 1) * DQ)
        ks = slice(g * D, (g + 1) * D)
        m = {
            "xt": np.ascontiguousarray(hidden_states[b].T).astype(BF),
            "wqkv": np.ascontiguousarray(
                np.concatenate([Wq[qs], Wk[ks], Wv[ks]], axis=0).T).astype(BF),
            "wg": np.ascontiguousarray(Wg[qs].T).astype(BF),
            "wo": np.ascontiguousarray(Wo[:, qs].T).astype(BF),
            "cwq": np.ascontiguousarray(cos[b] * qw * LAM),
            "swq": np.ascontiguousarray(sin[b] * (sign * qw_swap) * LAM),
            "cwk": np.ascontiguousarray(cos[b] * kw),
            "swk": np.ascontiguousarray(sin[b] * (sign * kw_swap)),
        }
        in_maps.append(m)

    res = run_bass_kernel_spmd(nc, in_maps, list(range(8)),
                               trace=bool(os.environ.get("BASS_TRACE")))
    LAST_EXEC_TIME_NS = res.exec_time_ns
    LAST_RESULTS = res

    out = np.empty((B, S, H), dtype=np.float32)
    for b in range(B):
        acc = res.results[4 * b]["y"].astype(np.float32)
        for g in range(1, 4):
            acc = acc + res.results[4 * b + g]["y"]
        out[b] = acc
    return out


# revision 43
# speedup vs baseline: 1.0356x; 1.0356x over previous
"""AFMoE attention layer on 8 NeuronCores (Trainium2, Bass/Tile).

Sharding: core c = (batch b = c//4) x (kv-head group g = c%4).
Each core computes its batch's q-heads 4g..4g+3 + kv head g end-to-end and a
partial output y_c = O_gated @ Wo[:, 512g:512(g+1)].T; the host sums the 4
group partials per batch (row-parallel Wo reduction done on host).

v2: bf16 operands on the PE (same 1 cycle/row as f32r, half the DMA/SBUF),
gate projection fused into the main projection loop (no second read of x),
software-pipelined attention (scores of task i+1 issued before PV of task i
so the PE never waits on the scalar-engine exp), sliding-window masking via
gpsimd affine_select on the exp'd scores (no mask matmuls, no mask DMA),
softmax denominator broadcast via gpsimd partition_broadcast, Wo matmuls
interleaved into the attention pipeline as PE filler, output DMA'd straight
from PSUM.
"""
import os

import ml_dtypes
import numpy as np

import concourse.bass as bass
import concourse.mybir as mybir
import concourse.tile as tile
from concourse.bass_utils import run_bass_kernel_spmd
from concourse.masks import make_identity

F32 = mybir.dt.float32
F32R = mybir.dt.float32r
BF16 = mybir.dt.bfloat16
AF = mybir.ActivationFunctionType
ALU = mybir.AluOpType
AX = mybir.AxisListType

B, S, H = 2, 2048, 2048
NH, NKV, D = 16, 4, 128
GROUPS = NH // NKV          # q heads per kv head = 4
QH = GROUPS                 # per-core q heads
DQ = QH * D                 # 512
EPS = 1e-5
NT = S // 128               # 16 s-tiles
NP = NT // 2                # 8 s-pairs (256 q rows each)
HC = H // 128               # 16 h-chunks
LAM = float(D) ** -0.5
NW = DQ + 2 * D             # 768
G = 2                       # k-blocks per PSUM score group (1 bank)

_nsplit = [0]


def _split_excess_waits(nc, limit=1):
    """This walrus build accepts only one semaphore wait per instruction
    (fp32/fp32r matmuls included). Move excess waits onto preceding
    same-engine NoOps; engine program order keeps this correct."""
    import bass_rust
    for blk in nc.m.functions[0].blocks:
        lst = blk.instructions
        idx = 0
        while idx < len(lst):
            inst = lst[idx]
            si = inst.sync_info
            if (si is None or len(si.on_wait) <= limit
                    or type(inst).__name__ == "InstCollectiveCompute"
                    or inst.engine == mybir.EngineType.Unassigned):
                idx += 1
                continue
            waits = list(si.on_wait)
            kept, excess = waits[-limit:], waits[:-limit]
            new_insts = []
            for w in excess:
                _nsplit[0] += 1
                nop = mybir.InstNoOp(name=f"WS-{_nsplit[0]}", ins=[], outs=[])
                nop.engine = inst.engine
                nop.sync_info = bass_rust.SyncInfo(on_wait=[w], on_update=[])
                new_insts.append(nop)
            inst.sync_info = bass_rust.SyncInfo(on_wait=kept,
                                                on_update=list(si.on_update))
            lst[idx:idx] = new_insts
            idx += len(new_insts) + 1


def _mask_plan(mask2d):
    """Classify the additive mask in [256(q) x 128(k)] slabs.

    Returns rows: rows[pair] = list of (kj, cuts) over a contiguous kj
    range, where cuts is a tuple of ('ge'|'le', base) affine-select specs
    in the transposed [k(part) x q(free)] layout: keep iff
    f - p + base {>=,<=} 0 for every cut.
    """
    rows = []
    fidx = np.arange(256)[None, :]
    pidx = np.arange(128)[:, None]
    diff = fidx - pidx                       # [128, 256]
    for p in range(NP):
        qsl = slice(p * 256, (p + 1) * 256)
        live = [kj for kj in range(NT)
                if (mask2d[qsl, kj * 128:(kj + 1) * 128] == 0.0).any()]
        if not live:
            raise ValueError("fully-masked query row block unsupported")
        lo, hi = min(live), max(live)
        row = []
        for kj in range(lo, hi + 1):
            blk = mask2d[qsl, kj * 128:(kj + 1) * 128]    # [256 q, 128 k]
            allowed = (blk == 0.0).T                      # [128 k, 256 q]
            if allowed.all():
                row.append((kj, ()))
                continue
            # allowed must be a function of (f - p) forming an interval
            dvals = np.unique(diff)
            adiff = {}
            ok = True
            for d in dvals:
                sel = allowed[diff == d]
                if sel.all():
                    adiff[d] = True
                elif not sel.any():
                    adiff[d] = False
                else:
                    ok = False
                    break
            live_d = sorted(d for d, a in adiff.items() if a)
            if not ok or not live_d:
                raise ValueError(f"non-affine mask block pair={p} kj={kj}")
            dlo, dhi = live_d[0], live_d[-1]
            if live_d != list(range(dlo, dhi + 1)):
                raise ValueError(f"non-interval mask block pair={p} kj={kj}")
            cuts = []
            if dlo > int(dvals.min()):
                cuts.append(('ge', -int(dlo)))
            if dhi < int(dvals.max()):
                cuts.append(('le', -int(dhi)))
            # verify
            keep = np.ones_like(allowed)
            for op, base in cuts:
                keep &= (diff + base >= 0) if op == 'ge' else (diff + base <= 0)
            if not (keep == allowed).all():
                raise ValueError(f"mask verify failed pair={p} kj={kj}")
            row.append((kj, tuple(cuts)))
        rows.append(row)
    return rows


def _build(rows):
    nc = bass.Bass()
    xt = nc.declare_dram_parameter("xt", [H, S], BF16, isOutput=False)
    wqkv = nc.declare_dram_parameter("wqkv", [H, NW], BF16, isOutput=False)
    wg = nc.declare_dram_parameter("wg", [H, DQ], BF16, isOutput=False)
    wo = nc.declare_dram_parameter("wo", [DQ, H], BF16, isOutput=False)
    cwq = nc.declare_dram_parameter("cwq", [S, D], F32, isOutput=False)
    swq = nc.declare_dram_parameter("swq", [S, D], F32, isOutput=False)
    cwk = nc.declare_dram_parameter("cwk", [S, D], F32, isOutput=False)
    swk = nc.declare_dram_parameter("swk", [S, D], F32, isOutput=False)
    y = nc.declare_dram_parameter("y", [S, H], F32, isOutput=True)
    dbg = bool(os.environ.get("KDBG"))
    if dbg:
        dbg_qT = nc.declare_dram_parameter("dbg_qT", [128, QH * S], BF16,
                                           isOutput=True)
        dbg_kT = nc.declare_dram_parameter("dbg_kT", [128, S], BF16,
                                           isOutput=True)
        dbg_v = nc.declare_dram_parameter("dbg_v", [128, NT * D], BF16,
                                          isOutput=True)
        dbg_sig = nc.declare_dram_parameter("dbg_sig", [128, QH * S], F32,
                                            isOutput=True)
        dbg_otg = nc.declare_dram_parameter("dbg_otg", [128, QH * S], BF16,
                                            isOutput=True)

    with tile.TileContext(nc) as tc, \
            nc.allow_low_precision(reason="bf16/fp32r matmul operands"), \
            tc.tile_pool(name="const", bufs=1) as const, \
            tc.tile_pool(name="persist", bufs=1) as pp, \
            tc.tile_pool(name="atw", bufs=1) as atw:
        identity_f = const.tile([128, 128], F32)
        make_identity(nc, identity_f)
        identity_r = const.tile([128, 128], F32R)
        nc.vector.tensor_copy(identity_r, identity_f)
        ones_mat_f = const.tile([128, 128], F32)
        nc.vector.memset(ones_mat_f, 1.0)
        ones_mat = const.tile([128, 128], BF16)
        nc.vector.tensor_copy(ones_mat, ones_mat_f)
        eps_t = const.tile([128, 1], F32)
        nc.vector.memset(eps_t, EPS)
        eps_dummy = const.tile([128, 1], F32)

        qT_all = pp.tile([128, QH, S], BF16)     # [d, h, s]
        kT_all = pp.tile([128, S], BF16)         # [d, s]
        v_all = pp.tile([128, NT, D], BF16)      # [s-part, s-tile, d]
        sigT_all = pp.tile([128, QH, S], F32)    # [d, m, s]
        OTg_all = pp.tile([128, QH, S], BF16)    # [d, h, s]
        wo_sb = atw.tile([128, QH, H], BF16)

        # ---------------- phase P-a: qkv + gate projections ---------------
        with tc.tile_pool(name="pw", bufs=1) as pw, \
                tc.tile_pool(name="pa", bufs=2) as pa, \
                tc.tile_pool(name="psa", bufs=1, space="PSUM") as psa:
            wqkv_sb = pw.tile([128, HC, NW], BF16)
            wg_sb = pw.tile([128, HC, DQ], BF16)
            xt4 = xt.rearrange("(c p) (t q) -> p c t q", p=128, q=256)

            def load_xt(pr, nsplit=4, eng=None):
                eng = eng or nc.sync
                t = pa.tile([128, HC, 256], BF16, tag="xt", bufs=2,
                            name=f"xt_p{pr}")
                for cq in range(HC // nsplit):
                    c0 = nsplit * cq
                    eng.dma_start(out=t[:, c0:c0 + nsplit, :],
                                  in_=xt4[:, c0:c0 + nsplit, pr, :])
                return t

            def load_tables(st):
                sl = slice(st * 128, (st + 1) * 128)
                cwq_t = pa.tile([128, D], F32, tag="cwq", bufs=4,
                                name=f"cwq_{st}")
                swq_t = pa.tile([128, D], F32, tag="swq", bufs=4,
                                name=f"swq_{st}")
                cwk_t = pa.tile([128, D], F32, tag="cwk", bufs=4,
                                name=f"cwk_{st}")
                swk_t = pa.tile([128, D], F32, tag="swk", bufs=4,
                                name=f"swk_{st}")
                nc.sync.dma_start(out=cwq_t, in_=cwq[sl, :])
                nc.sync.dma_start(out=swq_t, in_=swq[sl, :])
                nc.sync.dma_start(out=cwk_t, in_=cwk[sl, :])
                nc.sync.dma_start(out=swk_t, in_=swk[sl, :])
                return cwq_t, swq_t, cwk_t, swk_t

            # Startup is SP-sequencer issue-rate bound (~565ns per
            # dma_start), so spread the first-wave issues across the idle
            # scalar DGE queue: wqkv on sync, pair-0 x on scalar.  Pair-0
            # rope tables are hoisted before the wg loads so the first
            # rope chain isn't starved.  wo is trickled in later, mid-loop.
            for h in range(HC):
                nc.sync.dma_start(out=wqkv_sb[:, h, :],
                                  in_=wqkv[h * 128:(h + 1) * 128, :])
            xt_next = load_xt(0, eng=nc.scalar)
            tables = {0: load_tables(0), 1: load_tables(1)}
            for h in range(HC):
                nc.sync.dma_start(out=wg_sb[:, h, :],
                                  in_=wg[h * 128:(h + 1) * 128, :])

            for pr in range(NP):
                xt_t = xt_next
                ropes = {}
                for st in (2 * pr, 2 * pr + 1):
                    half = slice((st % 2) * 128, (st % 2) * 128 + 128)
                    cwq_t, swq_t, cwk_t, swk_t = (
                        tables.pop(st) if st in tables else load_tables(st))

                    pqkv = psa.tile([128, NW], F32, tag="pqkv", bufs=2)
                    for h in range(HC):
                        nc.tensor.matmul(pqkv[:, :DQ], xt_t[:, h, half],
                                         wqkv_sb[:, h, :DQ],
                                         start=(h == 0), stop=(h == HC - 1))
                        nc.tensor.matmul(pqkv[:, DQ:], xt_t[:, h, half],
                                         wqkv_sb[:, h, DQ:],
                                         start=(h == 0), stop=(h == HC - 1))
                    q_raw = pa.tile([128, DQ], F32, tag="qraw")
                    nc.scalar.copy(q_raw, pqkv[:, :DQ])
                    k_raw = pa.tile([128, D], F32, tag="kraw")
                    nc.scalar.copy(k_raw, pqkv[:, DQ:DQ + D])
                    nc.scalar.copy(v_all[:, st, :], pqkv[:, DQ + D:])

                    sq = pa.tile([128, DQ], F32, tag="sq")
                    nc.vector.tensor_mul(sq, q_raw, q_raw)
                    sqk = pa.tile([128, D], F32, tag="sqk")
                    nc.vector.tensor_mul(sqk, k_raw, k_raw)
                    ss = pa.tile([128, QH + 1], F32, tag="ss")
                    nc.vector.tensor_reduce(
                        ss[:, :QH], sq.rearrange("p (h d) -> p h d", d=D),
                        axis=AX.X, op=ALU.add)
                    nc.vector.tensor_reduce(ss[:, QH:], sqk,
                                            axis=AX.X, op=ALU.add)
                    rt = pa.tile([128, QH + 1], F32, tag="rt")
                    nc.scalar.activation(rt, ss, AF.Sqrt, bias=eps_t,
                                         scale=1.0 / D)
                    rq = pa.tile([128, QH + 1], F32, tag="rq")
                    nc.vector.reciprocal(rq, rt)
                    rk = rq[:, QH:QH + 1]

                    # rope swaps (half-rotations) of the raw values, on DVE
                    # (gpsimd CASTs are ~3x slower and add engine hops)
                    r_q = pa.tile([128, QH, D], F32R, tag="rqrot")
                    qv = q_raw.rearrange("p (h s d) -> p h s d", h=QH, s=2)
                    rv = r_q.rearrange("p h (s d) -> p h s d", s=2)
                    nc.vector.tensor_copy(rv[:, :, 0, :], qv[:, :, 1, :])
                    nc.vector.tensor_copy(rv[:, :, 1, :], qv[:, :, 0, :])
                    r_k = pa.tile([128, D], F32R, tag="rkrot")
                    nc.vector.tensor_copy(r_k[:, :64], k_raw[:, 64:])
                    nc.vector.tensor_copy(r_k[:, 64:], k_raw[:, :64])

                    qrope = pa.tile([128, QH, D], F32R, tag="qrope", bufs=3)
                    qh = q_raw.rearrange("p (h d) -> p h d", d=D)
                    for h in range(QH):
                        nc.vector.scalar_tensor_tensor(
                            qrope[:, h, :], qh[:, h, :], rq[:, h:h + 1], cwq_t,
                            op0=ALU.mult, op1=ALU.mult)
                        nc.vector.scalar_tensor_tensor(
                            r_q[:, h, :], r_q[:, h, :], rq[:, h:h + 1], swq_t,
                            op0=ALU.mult, op1=ALU.mult)
                    nc.gpsimd.tensor_tensor(qrope, qrope, r_q, op=ALU.add)

                    krope = pa.tile([128, D], F32R, tag="krope", bufs=3)
                    nc.vector.scalar_tensor_tensor(krope, k_raw, rk, cwk_t,
                                                   op0=ALU.mult, op1=ALU.mult)
                    nc.vector.scalar_tensor_tensor(r_k, r_k, rk, swk_t,
                                                   op0=ALU.mult, op1=ALU.mult)
                    nc.gpsimd.tensor_tensor(krope, krope, r_k, op=ALU.add)
                    ropes[st] = (qrope, krope)

                # prefetch next pair's x while this pair computes
                if pr + 1 < NP:
                    xt_next = load_xt(pr + 1)
                # trickle in wo during the middle of P-a
                if 2 <= pr <= 5:
                    dc = pr - 2
                    nc.sync.dma_start(out=wo_sb[:, dc, :H // 2],
                                      in_=wo[dc * 128:(dc + 1) * 128, :H // 2])
                    nc.sync.dma_start(out=wo_sb[:, dc, H // 2:],
                                      in_=wo[dc * 128:(dc + 1) * 128, H // 2:])

                def emit_transpose(st):
                    qrope, krope = ropes.pop(st)
                    sl = slice(st * 128, (st + 1) * 128)
                    ptqk = psa.tile([128, 5, 128], F32R, tag="pt", bufs=1)
                    for h in range(QH):
                        nc.tensor.transpose(ptqk[:, h, :], qrope[:, h, :],
                                            identity_r)
                    nc.tensor.transpose(ptqk[:, QH, :], krope, identity_r)
                    nc.scalar.copy(qT_all[:, :, sl], ptqk[:, :QH, :])
                    nc.scalar.copy(kT_all[:, sl], ptqk[:, QH, :])

                # PE order: qkv(st0) qkv(st1) gate transp(st0) transp(st1);
                # the ~7us rope chain (DVE/gpsimd) hides under qkv+gate.
                pg = psa.tile([128, QH, 256], F32, tag="pg", bufs=1)
                for m in range(QH):
                    for h in range(HC):
                        nc.tensor.matmul(
                            pg[:, m, :], wg_sb[:, h, m * 128:(m + 1) * 128],
                            xt_t[:, h, :],
                            start=(h == 0), stop=(h == HC - 1))
                qsl = slice(pr * 256, (pr + 1) * 256)
                nc.scalar.activation(sigT_all[:, :, qsl], pg, AF.Sigmoid)
                # dummy op to preload the sqrt act table off the rope
                # critical path (sqrt and sigmoid live in different tables)
                nc.scalar.activation(eps_dummy, eps_t, AF.Sqrt)

                emit_transpose(2 * pr)
                emit_transpose(2 * pr + 1)

        # ---------------- attention + gating + Wo -------------------------
        with tc.tile_pool(name="at", bufs=1) as at, \
                tc.tile_pool(name="ps_st", bufs=1, space="PSUM") as ps_st, \
                tc.tile_pool(name="ps_ot", bufs=1, space="PSUM") as ps_ot, \
                tc.tile_pool(name="ps_y", bufs=1, space="PSUM") as ps_y:
            # big pairs first to saturate the pipeline during warmup, but
            # end on big pairs too so the wo backlog drains before the tail
            pair_order = [NP - 1, NP - 2] + list(range(NP - 2))
            tasks = []
            for pr in pair_order:
                row = rows[pr]
                groups = [row[i:i + G] for i in range(0, len(row), G)]
                for h in range(QH):
                    for gi, grp in enumerate(groups):
                        tasks.append((pr, h, grp, gi == 0,
                                      gi == len(groups) - 1))

            ot_tiles = {}
            rs_tiles = {}
            est_tiles = {}
            wo_queue = []          # (ready_idx, st, n4)

            def emit_scores(idx, t):
                pr, h, grp, first, last = t
                qsl = slice(pr * 256, (pr + 1) * 256)
                ng = len(grp)
                st_ps = ps_st.tile([128, G, 256], F32, tag="st", bufs=3)
                for j, (kj, cuts) in enumerate(grp):
                    nc.tensor.matmul(
                        st_ps[:, j, :],
                        kT_all[:, kj * 128:(kj + 1) * 128],
                        qT_all[:, h, qsl],
                        start=True, stop=True)
                est = at.tile([128, G, 256], BF16, tag="est", bufs=4)
                nc.scalar.activation(
                    est[:, :ng, :].rearrange("p g q -> p (g q)"),
                    st_ps[:, :ng, :].rearrange("p g q -> p (g q)"),
                    AF.Exp)
                for j, (kj, cuts) in enumerate(grp):
                    for op, base in cuts:
                        # codegen only implements is_ge; express 'le' as the
                        # negated iota: f-p+base<=0  <=>  -f+p-base>=0
                        if op == 'ge':
                            pat, cm, b = [[1, 256]], -1, base
                        else:
                            pat, cm, b = [[-1, 256]], 1, -base
                        nc.gpsimd.affine_select(
                            est[:, j, :], est[:, j, :],
                            pattern=pat, compare_op=ALU.is_ge,
                            fill=0.0, base=b, channel_multiplier=cm)
                est_tiles[idx] = est

            def emit_pv(idx, t):
                pr, h, grp, first, last = t
                est = est_tiles.pop(idx)
                if first:
                    # ot and rowsum share one bank as ONE accumulation
                    # group: the first start=True zeroes the whole 2KB
                    # zero region (both halves), everything else joins
                    # with start=False
                    ot_tiles[(pr, h)] = ps_ot.tile([128, 512], F32,
                                                   tag="otrs", bufs=2,
                                                   name=f"otrs_{pr}_{h}")
                otrs = ot_tiles[(pr, h)]
                ot_t = otrs[:, 0:256]
                rs_t = otrs[:, 256:512]
                ng = len(grp)
                for j, (kj, cuts) in enumerate(grp):
                    stop = last and (j == ng - 1)
                    nc.tensor.matmul(ot_t, v_all[:, kj, :], est[:, j, :],
                                     start=(first and j == 0), stop=False)
                    # ones matrix: rowsum lands replicated on all partitions
                    nc.tensor.matmul(rs_t, ones_mat, est[:, j, :],
                                     start=False, stop=stop)

            def emit_norm(pr, h):
                qsl = slice(pr * 256, (pr + 1) * 256)
                otrs = ot_tiles.pop((pr, h))
                ot_t = otrs[:, 0:256]
                bcast = at.tile([128, 256], F32, tag="bcast", bufs=2)
                nc.vector.reciprocal(bcast, otrs[:, 256:512])
                sgr = at.tile([128, 256], F32, tag="sgr", bufs=2)
                nc.vector.tensor_mul(sgr, bcast, sigT_all[:, h, qsl])
                nc.vector.tensor_mul(OTg_all[:, h, qsl], ot_t, sgr)

            def emit_wo_chunk(st, n4):
                sl = slice(st * 128, (st + 1) * 128)
                py = ps_y.tile([128, 512], F32, tag="py", bufs=2)
                for dc in range(QH):
                    nc.tensor.matmul(
                        py, OTg_all[:, dc, sl],
                        wo_sb[:, dc, n4 * 512:(n4 + 1) * 512],
                        start=(dc == 0), stop=(dc == QH - 1))
                y_sb = at.tile([128, 512], F32, tag="ysb", bufs=2)
                # scalar engine: the DVE queue lags too much here
                nc.scalar.copy(y_sb, py)
                nc.sync.dma_start(out=y[sl, n4 * 512:(n4 + 1) * 512], in_=y_sb)

            LOOKAHEAD = 3

            def finish(pidx, pt, idx):
                emit_pv(pidx, pt)
                pr, h, grp, first, last = pt
                if last:
                    emit_norm(pr, h)
                    if h == QH - 1:
                        for st in (2 * pr, 2 * pr + 1):
                            for n4 in range(4):
                                wo_queue.append((idx, st, n4))
                for _ in range(2):
                    if wo_queue and idx - wo_queue[0][0] >= 3:
                        _, st, n4 = wo_queue.pop(0)
                        emit_wo_chunk(st, n4)

            pend = []
            for idx, t in enumerate(tasks):
                emit_scores(idx, t)
                pend.append((idx, t))
                if len(pend) > LOOKAHEAD:
                    pidx, pt = pend.pop(0)
                    finish(pidx, pt, idx)
            for pidx, pt in pend:
                finish(pidx, pt, len(tasks))
            for _, st, n4 in wo_queue:
                emit_wo_chunk(st, n4)

            if dbg:
                for t, dst in [(qT_all, dbg_qT), (kT_all, dbg_kT),
                               (v_all, dbg_v), (sigT_all, dbg_sig),
                               (OTg_all, dbg_otg)]:
                    nc.sync.dma_start(out=dst[:, :], in_=t)

    _split_excess_waits(nc)
    return nc


_CACHE = {}
LAST_EXEC_TIME_NS = None
LAST_RESULTS = None


def _maybe_install_profile_hook():
    if not os.environ.get("BASS_TRACE"):
        return
    try:
        import sys
        import types
        import antenv
        if "antenv.axon_hooks" in sys.modules:
            return
        mod = types.ModuleType("antenv.axon_hooks")
        mod._hook = None
        mod.set_axon_ntff_profile_hook = lambda h: setattr(mod, "_hook", h)
        mod.get_axon_ntff_profile_hook = lambda: mod._hook
        sys.modules["antenv.axon_hooks"] = mod
        antenv.axon_hooks = mod
        from trn_agent_boot.trn_boot import _ntff_profile_via_ctypes
        mod.set_axon_ntff_profile_hook(
            _ntff_profile_via_ctypes("/opt/axon/libaxon_pjrt.so"))
    except Exception:
        pass


def kernel(hidden_states, cos, sin, attention_mask, Wq, Wk, Wv, Wo, Wg,
           q_norm_w, k_norm_w):
    global LAST_EXEC_TIME_NS, LAST_RESULTS
    _maybe_install_profile_hook()

    BF = ml_dtypes.bfloat16
    hidden_states = np.asarray(hidden_states, dtype=np.float32)
    cos = np.asarray(cos, dtype=np.float32)
    sin = np.asarray(sin, dtype=np.float32)
    mask2d = np.asarray(attention_mask, dtype=np.float32).reshape(S, S)
    Wq = np.asarray(Wq, dtype=np.float32)
    Wk = np.asarray(Wk, dtype=np.float32)
    Wv = np.asarray(Wv, dtype=np.float32)
    Wo = np.asarray(Wo, dtype=np.float32)
    Wg = np.asarray(Wg, dtype=np.float32)
    qw = np.asarray(q_norm_w, dtype=np.float32)
    kw = np.asarray(k_norm_w, dtype=np.float32)

    rows = _mask_plan(mask2d)
    plan_key = tuple(tuple(r) for r in rows)
    if plan_key not in _CACHE:
        _CACHE[plan_key] = _build(rows)
    nc = _CACHE[plan_key]

    sign = np.concatenate([-np.ones(D // 2), np.ones(D // 2)]).astype(np.float32)
    qw_swap = np.concatenate([qw[D // 2:], qw[:D // 2]])
    kw_swap = np.concatenate([kw[D // 2:], kw[:D // 2]])

    in_maps = []
    for c in range(8):
        b, g = divmod(c, 4)
        qs = slice(g * DQ, (g + 1) * DQ)
        ks = slice(g * D, (g + 1) * D)
        m = {
            "xt": np.ascontiguousarray(hidden_states[b].T).astype(BF),
            "wqkv": np.ascontiguousarray(
                np.concatenate([Wq[qs], Wk[ks], Wv[ks]], axis=0).T).astype(BF),
            "wg": np.ascontiguousarray(Wg[qs].T).astype(BF),
            "wo": np.ascontiguousarray(Wo[:, qs].T).astype(BF),
            "cwq": np.ascontiguousarray(cos[b] * qw * LAM),
            "swq": np.ascontiguousarray(sin[b] * (sign * qw_swap) * LAM),
            "cwk": np.ascontiguousarray(cos[b] * kw),
            "swk": np.ascontiguousarray(sin[b] * (sign * kw_swap)),
        }
        in_maps.append(m)

    res = run_bass_kernel_spmd(nc, in_maps, list(range(8)),
                               trace=bool(os.environ.get("BASS_TRACE")))
    LAST_EXEC_TIME_NS = res.exec_time_ns
    LAST_RESULTS = res

    out = np.empty((B, S, H), dtype=np.float32)
    for b in range(B):
        acc = res.results[4 * b]["y"].astype(np.float32)
        for g in range(1, 4):
            acc = acc + res.results[4 * b + g]["y"]
        out[b] = acc
    return out


# revision 44
# speedup vs baseline: 1.0409x; 1.0051x over previous
"""AFMoE attention layer on 8 NeuronCores (Trainium2, Bass/Tile).

Sharding: core c = (batch b = c//4) x (kv-head group g = c%4).
Each core computes its batch's q-heads 4g..4g+3 + kv head g end-to-end and a
partial output y_c = O_gated @ Wo[:, 512g:512(g+1)].T; the host sums the 4
group partials per batch (row-parallel Wo reduction done on host).

v2: bf16 operands on the PE (same 1 cycle/row as f32r, half the DMA/SBUF),
gate projection fused into the main projection loop (no second read of x),
software-pipelined attention (scores of task i+1 issued before PV of task i
so the PE never waits on the scalar-engine exp), sliding-window masking via
gpsimd affine_select on the exp'd scores (no mask matmuls, no mask DMA),
softmax denominator broadcast via gpsimd partition_broadcast, Wo matmuls
interleaved into the attention pipeline as PE filler, output DMA'd straight
from PSUM.
"""
import os

import ml_dtypes
import numpy as np

import concourse.bass as bass
import concourse.mybir as mybir
import concourse.tile as tile
from concourse.bass_utils import run_bass_kernel_spmd
from concourse.masks import make_identity

F32 = mybir.dt.float32
F32R = mybir.dt.float32r
BF16 = mybir.dt.bfloat16
AF = mybir.ActivationFunctionType
ALU = mybir.AluOpType
AX = mybir.AxisListType

B, S, H = 2, 2048, 2048
NH, NKV, D = 16, 4, 128
GROUPS = NH // NKV          # q heads per kv head = 4
QH = GROUPS                 # per-core q heads
DQ = QH * D                 # 512
EPS = 1e-5
NT = S // 128               # 16 s-tiles
NP = NT // 2                # 8 s-pairs (256 q rows each)
HC = H // 128               # 16 h-chunks
LAM = float(D) ** -0.5
NW = DQ + 2 * D             # 768
G = 2                       # k-blocks per PSUM score group (1 bank)

_nsplit = [0]


def _split_excess_waits(nc, limit=1):
    """This walrus build accepts only one semaphore wait per instruction
    (fp32/fp32r matmuls included). Move excess waits onto preceding
    same-engine NoOps; engine program order keeps this correct."""
    import bass_rust
    for blk in nc.m.functions[0].blocks:
        lst = blk.instructions
        idx = 0
        while idx < len(lst):
            inst = lst[idx]
            si = inst.sync_info
            if (si is None or len(si.on_wait) <= limit
                    or type(inst).__name__ == "InstCollectiveCompute"
                    or inst.engine == mybir.EngineType.Unassigned):
                idx += 1
                continue
            waits = list(si.on_wait)
            kept, excess = waits[-limit:], waits[:-limit]
            new_insts = []
            for w in excess:
                _nsplit[0] += 1
                nop = mybir.InstNoOp(name=f"WS-{_nsplit[0]}", ins=[], outs=[])
                nop.engine = inst.engine
                nop.sync_info = bass_rust.SyncInfo(on_wait=[w], on_update=[])
                new_insts.append(nop)
            inst.sync_info = bass_rust.SyncInfo(on_wait=kept,
                                                on_update=list(si.on_update))
            lst[idx:idx] = new_insts
            idx += len(new_insts) + 1


def _mask_plan(mask2d):
    """Classify the additive mask in [256(q) x 128(k)] slabs.

    Returns rows: rows[pair] = list of (kj, cuts) over a contiguous kj
    range, where cuts is a tuple of ('ge'|'le', base) affine-select specs
    in the transposed [k(part) x q(free)] layout: keep iff
    f - p + base {>=,<=} 0 for every cut.
    """
    rows = []
    fidx = np.arange(256)[None, :]
    pidx = np.arange(128)[:, None]
    diff = fidx - pidx                       # [128, 256]
    for p in range(NP):
        qsl = slice(p * 256, (p + 1) * 256)
        live = [kj for kj in range(NT)
                if (mask2d[qsl, kj * 128:(kj + 1) * 128] == 0.0).any()]
        if not live:
            raise ValueError("fully-masked query row block unsupported")
        lo, hi = min(live), max(live)
        row = []
        for kj in range(lo, hi + 1):
            blk = mask2d[qsl, kj * 128:(kj + 1) * 128]    # [256 q, 128 k]
            allowed = (blk == 0.0).T                      # [128 k, 256 q]
            if allowed.all():
                row.append((kj, ()))
                continue
            # allowed must be a function of (f - p) forming an interval
            dvals = np.unique(diff)
            adiff = {}
            ok = True
            for d in dvals:
                sel = allowed[diff == d]
                if sel.all():
                    adiff[d] = True
                elif not sel.any():
                    adiff[d] = False
                else:
                    ok = False
                    break
            live_d = sorted(d for d, a in adiff.items() if a)
            if not ok or not live_d:
                raise ValueError(f"non-affine mask block pair={p} kj={kj}")
            dlo, dhi = live_d[0], live_d[-1]
            if live_d != list(range(dlo, dhi + 1)):
                raise ValueError(f"non-interval mask block pair={p} kj={kj}")
            cuts = []
            if dlo > int(dvals.min()):
                cuts.append(('ge', -int(dlo)))
            if dhi < int(dvals.max()):
                cuts.append(('le', -int(dhi)))
            # verify
            keep = np.ones_like(allowed)
            for op, base in cuts:
                keep &= (diff + base >= 0) if op == 'ge' else (diff + base <= 0)
            if not (keep == allowed).all():
                raise ValueError(f"mask verify failed pair={p} kj={kj}")
            row.append((kj, tuple(cuts)))
        rows.append(row)
    return rows


def _build(rows):
    nc = bass.Bass()
    xt = nc.declare_dram_parameter("xt", [H, S], BF16, isOutput=False)
    wqkv = nc.declare_dram_parameter("wqkv", [H, NW], BF16, isOutput=False)
    wg = nc.declare_dram_parameter("wg", [H, DQ], BF16, isOutput=False)
    wo = nc.declare_dram_parameter("wo", [DQ, H], BF16, isOutput=False)
    cwq = nc.declare_dram_parameter("cwq", [S, D], F32, isOutput=False)
    swq = nc.declare_dram_parameter("swq", [S, D], F32, isOutput=False)
    cwk = nc.declare_dram_parameter("cwk", [S, D], F32, isOutput=False)
    swk = nc.declare_dram_parameter("swk", [S, D], F32, isOutput=False)
    y = nc.declare_dram_parameter("y", [S, H], F32, isOutput=True)
    dbg = bool(os.environ.get("KDBG"))
    if dbg:
        dbg_qT = nc.declare_dram_parameter("dbg_qT", [128, QH * S], BF16,
                                           isOutput=True)
        dbg_kT = nc.declare_dram_parameter("dbg_kT", [128, S], BF16,
                                           isOutput=True)
        dbg_v = nc.declare_dram_parameter("dbg_v", [128, NT * D], BF16,
                                          isOutput=True)
        dbg_sig = nc.declare_dram_parameter("dbg_sig", [128, QH * S], F32,
                                            isOutput=True)
        dbg_otg = nc.declare_dram_parameter("dbg_otg", [128, QH * S], BF16,
                                            isOutput=True)

    with tile.TileContext(nc) as tc, \
            nc.allow_low_precision(reason="bf16/fp32r matmul operands"), \
            tc.tile_pool(name="const", bufs=1) as const, \
            tc.tile_pool(name="persist", bufs=1) as pp, \
            tc.tile_pool(name="atw", bufs=1) as atw:
        identity_f = const.tile([128, 128], F32)
        make_identity(nc, identity_f)
        identity_r = const.tile([128, 128], F32R)
        nc.vector.tensor_copy(identity_r, identity_f)
        ones_mat_f = const.tile([128, 128], F32)
        nc.vector.memset(ones_mat_f, 1.0)
        ones_mat = const.tile([128, 128], BF16)
        nc.vector.tensor_copy(ones_mat, ones_mat_f)
        eps_t = const.tile([128, 1], F32)
        nc.vector.memset(eps_t, EPS)
        eps_dummy = const.tile([128, 1], F32)

        qT_all = pp.tile([128, QH, S], BF16)     # [d, h, s]
        kT_all = pp.tile([128, S], BF16)         # [d, s]
        v_all = pp.tile([128, NT, D], BF16)      # [s-part, s-tile, d]
        sigT_all = pp.tile([128, QH, S], F32)    # [d, m, s]
        OTg_all = pp.tile([128, QH, S], BF16)    # [d, h, s]
        wo_sb = atw.tile([128, QH, H], BF16)

        # ---------------- phase P-a: qkv + gate projections ---------------
        with tc.tile_pool(name="pw", bufs=1) as pw, \
                tc.tile_pool(name="pa", bufs=2) as pa, \
                tc.tile_pool(name="psa", bufs=1, space="PSUM") as psa:
            wqkv_sb = pw.tile([128, HC, NW], BF16)
            wg_sb = pw.tile([128, HC, DQ], BF16)
            xt4 = xt.rearrange("(c p) (t q) -> p c t q", p=128, q=256)

            def load_xt(pr, nsplit=4, eng=None):
                eng = eng or nc.sync
                t = pa.tile([128, HC, 256], BF16, tag="xt", bufs=2,
                            name=f"xt_p{pr}")
                for cq in range(HC // nsplit):
                    c0 = nsplit * cq
                    eng.dma_start(out=t[:, c0:c0 + nsplit, :],
                                  in_=xt4[:, c0:c0 + nsplit, pr, :])
                return t

            def load_tables(st):
                sl = slice(st * 128, (st + 1) * 128)
                cwq_t = pa.tile([128, D], F32, tag="cwq", bufs=4,
                                name=f"cwq_{st}")
                swq_t = pa.tile([128, D], F32, tag="swq", bufs=4,
                                name=f"swq_{st}")
                cwk_t = pa.tile([128, D], F32, tag="cwk", bufs=4,
                                name=f"cwk_{st}")
                swk_t = pa.tile([128, D], F32, tag="swk", bufs=4,
                                name=f"swk_{st}")
                nc.sync.dma_start(out=cwq_t, in_=cwq[sl, :])
                nc.sync.dma_start(out=swq_t, in_=swq[sl, :])
                nc.sync.dma_start(out=cwk_t, in_=cwk[sl, :])
                nc.sync.dma_start(out=swk_t, in_=swk[sl, :])
                return cwq_t, swq_t, cwk_t, swk_t

            # Startup is SP-sequencer issue-rate bound (~565ns per
            # dma_start), so spread the first-wave issues across the idle
            # scalar DGE queue: wqkv on sync, pair-0 x on scalar.  Pair-0
            # rope tables are hoisted before the wg loads so the first
            # rope chain isn't starved.  wo is trickled in later, mid-loop.
            for h in range(HC):
                nc.sync.dma_start(out=wqkv_sb[:, h, :],
                                  in_=wqkv[h * 128:(h + 1) * 128, :])
            xt_next = load_xt(0, eng=nc.scalar)
            tables = {0: load_tables(0), 1: load_tables(1)}
            for h in range(HC):
                eng = nc.sync if h % 2 == 0 else nc.scalar
                eng.dma_start(out=wg_sb[:, h, :],
                              in_=wg[h * 128:(h + 1) * 128, :])

            for pr in range(NP):
                xt_t = xt_next
                ropes = {}
                for st in (2 * pr, 2 * pr + 1):
                    half = slice((st % 2) * 128, (st % 2) * 128 + 128)
                    cwq_t, swq_t, cwk_t, swk_t = (
                        tables.pop(st) if st in tables else load_tables(st))

                    pqkv = psa.tile([128, NW], F32, tag="pqkv", bufs=2)
                    for h in range(HC):
                        nc.tensor.matmul(pqkv[:, :DQ], xt_t[:, h, half],
                                         wqkv_sb[:, h, :DQ],
                                         start=(h == 0), stop=(h == HC - 1))
                        nc.tensor.matmul(pqkv[:, DQ:], xt_t[:, h, half],
                                         wqkv_sb[:, h, DQ:],
                                         start=(h == 0), stop=(h == HC - 1))
                    q_raw = pa.tile([128, DQ], F32, tag="qraw")
                    nc.scalar.copy(q_raw, pqkv[:, :DQ])
                    k_raw = pa.tile([128, D], F32, tag="kraw")
                    nc.scalar.copy(k_raw, pqkv[:, DQ:DQ + D])
                    nc.scalar.copy(v_all[:, st, :], pqkv[:, DQ + D:])

                    sq = pa.tile([128, DQ], F32, tag="sq")
                    nc.vector.tensor_mul(sq, q_raw, q_raw)
                    sqk = pa.tile([128, D], F32, tag="sqk")
                    nc.vector.tensor_mul(sqk, k_raw, k_raw)
                    ss = pa.tile([128, QH + 1], F32, tag="ss")
                    nc.vector.tensor_reduce(
                        ss[:, :QH], sq.rearrange("p (h d) -> p h d", d=D),
                        axis=AX.X, op=ALU.add)
                    nc.vector.tensor_reduce(ss[:, QH:], sqk,
                                            axis=AX.X, op=ALU.add)
                    rt = pa.tile([128, QH + 1], F32, tag="rt")
                    nc.scalar.activation(rt, ss, AF.Sqrt, bias=eps_t,
                                         scale=1.0 / D)
                    rq = pa.tile([128, QH + 1], F32, tag="rq")
                    nc.vector.reciprocal(rq, rt)
                    rk = rq[:, QH:QH + 1]

                    # rope swaps (half-rotations) of the raw values, on DVE
                    # (gpsimd CASTs are ~3x slower and add engine hops)
                    r_q = pa.tile([128, QH, D], F32R, tag="rqrot")
                    qv = q_raw.rearrange("p (h s d) -> p h s d", h=QH, s=2)
                    rv = r_q.rearrange("p h (s d) -> p h s d", s=2)
                    nc.vector.tensor_copy(rv[:, :, 0, :], qv[:, :, 1, :])
                    nc.vector.tensor_copy(rv[:, :, 1, :], qv[:, :, 0, :])
                    r_k = pa.tile([128, D], F32R, tag="rkrot")
                    nc.vector.tensor_copy(r_k[:, :64], k_raw[:, 64:])
                    nc.vector.tensor_copy(r_k[:, 64:], k_raw[:, :64])

                    qrope = pa.tile([128, QH, D], F32R, tag="qrope", bufs=3)
                    qh = q_raw.rearrange("p (h d) -> p h d", d=D)
                    for h in range(QH):
                        nc.vector.scalar_tensor_tensor(
                            qrope[:, h, :], qh[:, h, :], rq[:, h:h + 1], cwq_t,
                            op0=ALU.mult, op1=ALU.mult)
                        nc.vector.scalar_tensor_tensor(
                            r_q[:, h, :], r_q[:, h, :], rq[:, h:h + 1], swq_t,
                            op0=ALU.mult, op1=ALU.mult)
                    nc.gpsimd.tensor_tensor(qrope, qrope, r_q, op=ALU.add)

                    krope = pa.tile([128, D], F32R, tag="krope", bufs=3)
                    nc.vector.scalar_tensor_tensor(krope, k_raw, rk, cwk_t,
                                                   op0=ALU.mult, op1=ALU.mult)
                    nc.vector.scalar_tensor_tensor(r_k, r_k, rk, swk_t,
                                                   op0=ALU.mult, op1=ALU.mult)
                    nc.gpsimd.tensor_tensor(krope, krope, r_k, op=ALU.add)
                    ropes[st] = (qrope, krope)

                # prefetch next pair's x while this pair computes
                if pr + 1 < NP:
                    xt_next = load_xt(pr + 1)
                # trickle in wo during the middle of P-a
                if 2 <= pr <= 5:
                    dc = pr - 2
                    nc.sync.dma_start(out=wo_sb[:, dc, :H // 2],
                                      in_=wo[dc * 128:(dc + 1) * 128, :H // 2])
                    nc.sync.dma_start(out=wo_sb[:, dc, H // 2:],
                                      in_=wo[dc * 128:(dc + 1) * 128, H // 2:])

                def emit_transpose(st):
                    qrope, krope = ropes.pop(st)
                    sl = slice(st * 128, (st + 1) * 128)
                    ptqk = psa.tile([128, 5, 128], F32R, tag="pt", bufs=1)
                    for h in range(QH):
                        nc.tensor.transpose(ptqk[:, h, :], qrope[:, h, :],
                                            identity_r)
                    nc.tensor.transpose(ptqk[:, QH, :], krope, identity_r)
                    nc.scalar.copy(qT_all[:, :, sl], ptqk[:, :QH, :])
                    nc.scalar.copy(kT_all[:, sl], ptqk[:, QH, :])

                # PE order: qkv(st0) qkv(st1) gate transp(st0) transp(st1);
                # the ~7us rope chain (DVE/gpsimd) hides under qkv+gate.
                pg = psa.tile([128, QH, 256], F32, tag="pg", bufs=1)
                for m in range(QH):
                    for h in range(HC):
                        nc.tensor.matmul(
                            pg[:, m, :], wg_sb[:, h, m * 128:(m + 1) * 128],
                            xt_t[:, h, :],
                            start=(h == 0), stop=(h == HC - 1))
                qsl = slice(pr * 256, (pr + 1) * 256)
                nc.scalar.activation(sigT_all[:, :, qsl], pg, AF.Sigmoid)
                # dummy op to preload the sqrt act table off the rope
                # critical path (sqrt and sigmoid live in different tables)
                nc.scalar.activation(eps_dummy, eps_t, AF.Sqrt)

                emit_transpose(2 * pr)
                emit_transpose(2 * pr + 1)

        # ---------------- attention + gating + Wo -------------------------
        with tc.tile_pool(name="at", bufs=1) as at, \
                tc.tile_pool(name="ps_st", bufs=1, space="PSUM") as ps_st, \
                tc.tile_pool(name="ps_ot", bufs=1, space="PSUM") as ps_ot, \
                tc.tile_pool(name="ps_y", bufs=1, space="PSUM") as ps_y:
            # big pairs first to saturate the pipeline during warmup, but
            # end on big pairs too so the wo backlog drains before the tail
            pair_order = [NP - 1, NP - 2] + list(range(NP - 2))
            tasks = []
            for pr in pair_order:
                row = rows[pr]
                groups = [row[i:i + G] for i in range(0, len(row), G)]
                for h in range(QH):
                    for gi, grp in enumerate(groups):
                        tasks.append((pr, h, grp, gi == 0,
                                      gi == len(groups) - 1))

            ot_tiles = {}
            rs_tiles = {}
            est_tiles = {}
            wo_queue = []          # (ready_idx, st, n4)

            def emit_scores(idx, t):
                pr, h, grp, first, last = t
                qsl = slice(pr * 256, (pr + 1) * 256)
                ng = len(grp)
                st_ps = ps_st.tile([128, G, 256], F32, tag="st", bufs=3)
                for j, (kj, cuts) in enumerate(grp):
                    nc.tensor.matmul(
                        st_ps[:, j, :],
                        kT_all[:, kj * 128:(kj + 1) * 128],
                        qT_all[:, h, qsl],
                        start=True, stop=True)
                est = at.tile([128, G, 256], BF16, tag="est", bufs=5)
                nc.scalar.activation(
                    est[:, :ng, :].rearrange("p g q -> p (g q)"),
                    st_ps[:, :ng, :].rearrange("p g q -> p (g q)"),
                    AF.Exp)
                for j, (kj, cuts) in enumerate(grp):
                    for op, base in cuts:
                        # codegen only implements is_ge; express 'le' as the
                        # negated iota: f-p+base<=0  <=>  -f+p-base>=0
                        if op == 'ge':
                            pat, cm, b = [[1, 256]], -1, base
                        else:
                            pat, cm, b = [[-1, 256]], 1, -base
                        nc.gpsimd.affine_select(
                            est[:, j, :], est[:, j, :],
                            pattern=pat, compare_op=ALU.is_ge,
                            fill=0.0, base=b, channel_multiplier=cm)
                est_tiles[idx] = est

            def emit_pv(idx, t):
                pr, h, grp, first, last = t
                est = est_tiles.pop(idx)
                if first:
                    # ot and rowsum share one bank as ONE accumulation
                    # group: the first start=True zeroes the whole 2KB
                    # zero region (both halves), everything else joins
                    # with start=False
                    ot_tiles[(pr, h)] = ps_ot.tile([128, 512], F32,
                                                   tag="otrs", bufs=2,
                                                   name=f"otrs_{pr}_{h}")
                otrs = ot_tiles[(pr, h)]
                ot_t = otrs[:, 0:256]
                rs_t = otrs[:, 256:512]
                ng = len(grp)
                for j, (kj, cuts) in enumerate(grp):
                    stop = last and (j == ng - 1)
                    nc.tensor.matmul(ot_t, v_all[:, kj, :], est[:, j, :],
                                     start=(first and j == 0), stop=False)
                    # ones matrix: rowsum lands replicated on all partitions
                    nc.tensor.matmul(rs_t, ones_mat, est[:, j, :],
                                     start=False, stop=stop)

            def emit_norm(pr, h):
                qsl = slice(pr * 256, (pr + 1) * 256)
                otrs = ot_tiles.pop((pr, h))
                ot_t = otrs[:, 0:256]
                bcast = at.tile([128, 256], F32, tag="bcast", bufs=2)
                nc.vector.reciprocal(bcast, otrs[:, 256:512])
                sgr = at.tile([128, 256], F32, tag="sgr", bufs=2)
                nc.vector.tensor_mul(sgr, bcast, sigT_all[:, h, qsl])
                nc.vector.tensor_mul(OTg_all[:, h, qsl], ot_t, sgr)

            def emit_wo_chunk(st, n4):
                sl = slice(st * 128, (st + 1) * 128)
                py = ps_y.tile([128, 512], F32, tag="py", bufs=2)
                for dc in range(QH):
                    nc.tensor.matmul(
                        py, OTg_all[:, dc, sl],
                        wo_sb[:, dc, n4 * 512:(n4 + 1) * 512],
                        start=(dc == 0), stop=(dc == QH - 1))
                y_sb = at.tile([128, 512], F32, tag="ysb", bufs=2)
                # scalar engine: the DVE queue lags too much here
                nc.scalar.copy(y_sb, py)
                nc.sync.dma_start(out=y[sl, n4 * 512:(n4 + 1) * 512], in_=y_sb)

            LOOKAHEAD = 4

            def finish(pidx, pt, idx):
                emit_pv(pidx, pt)
                pr, h, grp, first, last = pt
                if last:
                    emit_norm(pr, h)
                    if h == QH - 1:
                        for st in (2 * pr, 2 * pr + 1):
                            for n4 in range(4):
                                wo_queue.append((idx, st, n4))
                for _ in range(2):
                    if wo_queue and idx - wo_queue[0][0] >= 3:
                        _, st, n4 = wo_queue.pop(0)
                        emit_wo_chunk(st, n4)

            pend = []
            for idx, t in enumerate(tasks):
                emit_scores(idx, t)
                pend.append((idx, t))
                if len(pend) > LOOKAHEAD:
                    pidx, pt = pend.pop(0)
                    finish(pidx, pt, idx)
            for pidx, pt in pend:
                finish(pidx, pt, len(tasks))
            for _, st, n4 in wo_queue:
                emit_wo_chunk(st, n4)

            if dbg:
                for t, dst in [(qT_all, dbg_qT), (kT_all, dbg_kT),
                               (v_all, dbg_v), (sigT_all, dbg_sig),
                               (OTg_all, dbg_otg)]:
                    nc.sync.dma_start(out=dst[:, :], in_=t)

    _split_excess_waits(nc)
    return nc


_CACHE = {}
LAST_EXEC_TIME_NS = None
LAST_RESULTS = None


def _maybe_install_profile_hook():
    if not os.environ.get("BASS_TRACE"):
        return
    try:
        import sys
        import types
        import antenv
        if "antenv.axon_hooks" in sys.modules:
            return
        mod = types.ModuleType("antenv.axon_hooks")
        mod._hook = None
        mod.set_axon_ntff_profile_hook = lambda h: setattr(mod, "_hook", h)
        mod.get_axon_ntff_profile_hook = lambda: mod._hook
        sys.modules["antenv.axon_hooks"] = mod
        antenv.axon_hooks = mod
        from trn_agent_boot.trn_boot import _ntff_profile_via_ctypes
        mod.set_axon_ntff_profile_hook(
            _ntff_profile_via_ctypes("/opt/axon/libaxon_pjrt.so"))
    except Exception:
        pass


def kernel(hidden_states, cos, sin, attention_mask, Wq, Wk, Wv, Wo, Wg,
           q_norm_w, k_norm_w):
    global LAST_EXEC_TIME_NS, LAST_RESULTS
    _maybe_install_profile_hook()

    BF = ml_dtypes.bfloat16
    hidden_states = np.asarray(hidden_states, dtype=np.float32)
    cos = np.asarray(cos, dtype=np.float32)
    sin = np.asarray(sin, dtype=np.float32)
    mask2d = np.asarray(attention_mask, dtype=np.float32).reshape(S, S)
    Wq = np.asarray(Wq, dtype=np.float32)
    Wk = np.asarray(Wk, dtype=np.float32)
    Wv = np.asarray(Wv, dtype=np.float32)
    Wo = np.asarray(Wo, dtype=np.float32)
    Wg = np.asarray(Wg, dtype=np.float32)
    qw = np.asarray(q_norm_w, dtype=np.float32)
    kw = np.asarray(k_norm_w, dtype=np.float32)

    rows = _mask_plan(mask2d)
    plan_key = tuple(tuple(r) for r in rows)
    if plan_key not in _CACHE:
        _CACHE[plan_key] = _build(rows)
    nc = _CACHE[plan_key]

    sign = np.concatenate([-np.ones(D // 2), np.ones(D // 2)]).astype(np.float32)
    qw_swap = np.concatenate([qw[D // 2:], qw[:D // 2]])
    kw_swap = np.concatenate([kw[D // 2:], kw[:D // 2]])

    in_maps = []
    for c in range(8):
        b, g = divmod(c, 4)
        qs = slice(g * DQ, (g + 1) * DQ)
        ks = slice(g * D, (g + 1) * D)
        m = {
            "xt": np.ascontiguousarray(hidden_states[b].T).astype(BF),
            "wqkv": np.ascontiguousarray(
                np.concatenate([Wq[qs], Wk[ks], Wv[ks]], axis=0).T).astype(BF),
            "wg": np.ascontiguousarray(Wg[qs].T).astype(BF),
            "wo": np.ascontiguousarray(Wo[:, qs].T).astype(BF),
            "cwq": np.ascontiguousarray(cos[b] * qw * LAM),
            "swq": np.ascontiguousarray(sin[b] * (sign * qw_swap) * LAM),
            "cwk": np.ascontiguousarray(cos[b] * kw),
            "swk": np.ascontiguousarray(sin[b] * (sign * kw_swap)),
        }
        in_maps.append(m)

    res = run_bass_kernel_spmd(nc, in_maps, list(range(8)),
                               trace=bool(os.environ.get("BASS_TRACE")))
    LAST_EXEC_TIME_NS = res.exec_time_ns
    LAST_RESULTS = res

    out = np.empty((B, S, H), dtype=np.float32)
    for b in range(B):
        acc = res.results[4 * b]["y"].astype(np.float32)
        for g in range(1, 4):
            acc = acc + res.results[4 * b + g]["y"]
        out[b] = acc
    return out
